# revision 54
# baseline (speedup 1.0000x reference)
"""Attention-pooling Trainium2 kernel, v6 (fp8 end-to-end).

Per-core (8 cores = 4 batches x 2 query-row halves): [2048, 4096] score
block via fp8 DoubleRow matmuls (256-contraction in one pass). exp
splits between ACT (exact exp -> e4m3, fused rowsum accumulator; a
softmax-invariant -2 shift keeps E inside e4m3 range) and DVE
(Schraudolph uint8 bit-trick writing e4m3 bit patterns straight from
the fp32 PSUM scores; saturating u8 conversion clamps the underflow
side to +0). Rowsums for DVE chunks ride one strided tensor_scalar
pass with fused accumulator. Colsums pair adjacent q-tiles through
DoubleRow (Es stored as [128, 2, S] pairs) with the M=8 sliding-window
trick: lhsT = [128,2,16] zero tile with recb*1024 (e4m3) in col 8, so
key-chunk m of a pair lands on psum partition m; chunks ping-pong two
PSUM regions so consecutive accumulations overlap. GPSIMD can read
neither PSUM nor reduce along the free axis, so it only issues DMAs
and casts recb.

Host: Q/K projections + fp8 quantization in, (w @ x) @ Wv finish out.
"""

import numpy as np

import concourse.bass as bass  # noqa: F401
import concourse.mybir as mybir
import concourse.tile as tile
from concourse import bacc

B, S, E = 4, 4096, 256
HALF = S // 2
P = 128
N_CORES = 8
QTILES = HALF // P     # 16
F32 = mybir.dt.float32
FP8 = mybir.dt.float8e4
BF16 = mybir.dt.bfloat16
U16 = mybir.dt.uint16
U8 = mybir.dt.uint8
ALU = mybir.AluOpType

EXPW = 1024            # psum chunk width
NEXP = S // EXPW       # 4 chunks per q-tile

LOG2E = float(np.log2(np.e))
CSHIFT = 2.0                          # exp(score/16 - CSHIFT): keeps E in fp8 range
SCH_A = 8.0 * LOG2E / 16.0            # Schraudolph scale (raw scores -> e4m3 bits)
SCH_B = 56.0 - 8.0 * CSHIFT * LOG2E - 0.5   # bias, -0.5 = sawtooth calibration
RECB_SCALE = 1024.0                   # recb prescale (undone on host)

# per-(tile, chunk) exp engine: A=ACT exact exp, V=DVE Schraudolph
SCHED = ["VAVA", "VAAA", "VVAA", "VAAA", "VVAA", "VAAA", "VVAA", "VAAA",
         "VVAA", "VAAA", "VVAA", "VAAA", "VVAA", "VAAA", "VVAA", "VAAA"]
COLSUM_LAG = 2  # tiles between exp emission and tile colsum emission
NOACC = set()  # disabled: the serial GPSIMD add-tree stalled the pipeline
NLT = 6         # recb lhsT ring depth (must exceed COLSUM_LAG + 1)


def _runs(pat):
    """Contiguous 'V' chunk runs [(start, len)] for one tile pattern."""
    out = []
    c = 0
    while c < NEXP:
        if pat[c] == "A":
            c += 1
            continue
        c0 = c
        while c < NEXP and pat[c] != "A":
            c += 1
        out.append((c0, c - c0))
    return out


def _emit(ctx, tc):
    nc = tc.nc

    qt_d = nc.dram_tensor("qt8", [P, 2, HALF], FP8, kind="ExternalInput")
    kt_d = nc.dram_tensor("kt8", [P, 2, S], FP8, kind="ExternalInput")
    w_d = nc.dram_tensor("w", [8, 1024], F32, kind="ExternalOutput")

    const = ctx.enter_context(tc.tile_pool(name="const", bufs=1))
    epool = ctx.enter_context(tc.tile_pool(name="epool", bufs=5))
    rsp = ctx.enter_context(tc.tile_pool(name="rsp", bufs=4))
    jkp = ctx.enter_context(tc.tile_pool(name="jkp", bufs=2))
    pp = ctx.enter_context(tc.tile_pool(name="pp", bufs=3, space="PSUM"))
    wp = ctx.enter_context(tc.tile_pool(name="wp", bufs=1, space="PSUM"))

    qt_sb = const.tile([P, 2, HALF], FP8, name="qt_sb")
    kt_sb = const.tile([P, 2, S], FP8, name="kt_sb")
    bias_t = const.tile([P, 1], F32, name="bias_t")
    nc.vector.memset(bias_t, -CSHIFT)
    lt = [const.tile([P, 2, 16], FP8, name=f"lt{i}") for i in range(NLT)]
    for i in range(NLT):
        nc.vector.memset(lt[i], 0.0)
    w_sb = const.tile([8, 1024], F32, name="w_sb")

    # ---- input DMAs: tiny first-needed slices, then bulk on 3 queues
    # first matmul needs kt[0:512] + qt[0:128]: split across parallel queues
    nc.scalar.dma_start(out=kt_sb[:, :, 0:288], in_=kt_d[:, :, 0:288])
    nc.sync.dma_start(out=kt_sb[:, :, 288:512], in_=kt_d[:, :, 288:512])
    nc.gpsimd.dma_start(out=qt_sb[:, :, 0:128], in_=qt_d[:, :, 0:128])
    nc.scalar.dma_start(out=kt_sb[:, :, 512:1024], in_=kt_d[:, :, 512:1024])
    nc.sync.dma_start(out=kt_sb[:, :, 1024:1536], in_=kt_d[:, :, 1024:1536])
    nc.gpsimd.dma_start(out=kt_sb[:, :, 1536:2048], in_=kt_d[:, :, 1536:2048])
    nc.scalar.dma_start(out=kt_sb[:, :, 2048:2560], in_=kt_d[:, :, 2048:2560])
    nc.sync.dma_start(out=kt_sb[:, :, 2560:3072], in_=kt_d[:, :, 2560:3072])
    nc.gpsimd.dma_start(out=kt_sb[:, :, 3072:3584], in_=kt_d[:, :, 3072:3584])
    nc.scalar.dma_start(out=kt_sb[:, :, 3584:4096], in_=kt_d[:, :, 3584:4096])
    nc.sync.dma_start(out=qt_sb[:, :, 128:256], in_=qt_d[:, :, 128:256])
    nc.scalar.dma_start(out=qt_sb[:, :, 256:512], in_=qt_d[:, :, 256:512])
    nc.sync.dma_start(out=qt_sb[:, :, 512:1024], in_=qt_d[:, :, 512:1024])
    nc.gpsimd.dma_start(out=qt_sb[:, :, 1024:2048], in_=qt_d[:, :, 1024:2048])

    w_ps = [wp.tile([8, 512], F32, name=f"w_ps{r}") for r in range(2)]
    es_tiles = {}
    lt_of = {}
    pending = []  # deferred colsum matmul thunks, drained ~1 per chunk

    def drain_pending(n):
        for _ in range(min(n, len(pending))):
            pending.pop(0)()

    def emit_tile(qi):
        j = qi % 2
        pair = qi // 2
        if j == 0:
            es_tiles[pair] = epool.tile([P, 2, S], FP8, tag="E", name=f"E{pair}")
            lt_of[pair] = lt[pair % NLT]
        Es = es_tiles[pair]
        pat = SCHED[qi]
        rs4 = rsp.tile([P, 4], F32, tag="rs4", name=f"rs4_{qi}")
        slot = 0
        for c in range(NEXP):
            ps = pp.tile([P, EXPW], F32, tag="ps", name=f"ps{qi}_{c}")
            for h in range(2):
                t0 = c * EXPW + h * 512
                nc.tensor.matmul(
                    ps[:, h * 512:(h + 1) * 512],
                    qt_sb[:, :, qi * P:(qi + 1) * P],
                    kt_sb[:, :, t0:t0 + 512],
                    start=True, stop=True,
                    perf_mode=mybir.MatmulPerfMode.DoubleRow,
                )
            drain_pending(2)
            dst = Es[:, j, c * EXPW:(c + 1) * EXPW]
            if pat[c] == "A":
                nc.scalar.activation(
                    out=dst, in_=ps,
                    func=mybir.ActivationFunctionType.Exp,
                    scale=1.0 / 16.0, bias=bias_t,
                    accum_out=None if qi in NOACC else rs4[:, slot:slot + 1],
                )
                if qi not in NOACC:
                    slot += 1
            else:
                nc.vector.tensor_scalar(
                    out=dst.bitcast(U8), in0=ps,
                    scalar1=SCH_A, scalar2=SCH_B,
                    op0=ALU.mult, op1=ALU.add,
                )
        # rowsum for V chunks: one DVE pass with fused accum; non-adjacent
        # V chunks are covered by a strided access pattern
        rsum = rsp.tile([P, 1], F32, tag="rsum", name=f"rsum{qi}")
        vcs = [c for c in range(NEXP) if pat[c] == "V"]
        if qi in NOACC:
            # GPSIMD add-tree over all four fp8 chunks -> one DVE reduce
            t1 = jkp.tile([P, EXPW], BF16, tag="half", name=f"t1_{qi}")
            nc.gpsimd.tensor_tensor(
                out=t1, in0=Es[:, j, 0:EXPW], in1=Es[:, j, EXPW:2 * EXPW],
                op=ALU.add)
            t2 = jkp.tile([P, EXPW], BF16, tag="jk", name=f"t2_{qi}")
            nc.gpsimd.tensor_tensor(
                out=t2, in0=Es[:, j, 2 * EXPW:3 * EXPW],
                in1=Es[:, j, 3 * EXPW:4 * EXPW], op=ALU.add)
            t3 = jkp.tile([P, EXPW], BF16, tag="t3", name=f"t3_{qi}")
            nc.gpsimd.tensor_tensor(out=t3, in0=t1, in1=t2, op=ALU.add)
            nc.vector.tensor_scalar(
                out=t1, in0=t3,
                scalar1=1.0, scalar2=0.0,
                op0=ALU.mult, op1=ALU.add,
                accum_out=rs4[:, 0:1],
            )
            slot = 1
        elif len(vcs) == 2:
            # GPSIMD pre-adds the two fp8 V chunks; DVE reduces the half
            half = jkp.tile([P, EXPW], BF16, tag="half", name=f"half{qi}")
            nc.gpsimd.tensor_tensor(
                out=half,
                in0=Es[:, j, vcs[0] * EXPW:(vcs[0] + 1) * EXPW],
                in1=Es[:, j, vcs[1] * EXPW:(vcs[1] + 1) * EXPW],
                op=ALU.add,
            )
            junk = jkp.tile([P, EXPW], BF16, tag="jk", name=f"jk{qi}")
            nc.vector.tensor_scalar(
                out=junk, in0=half,
                scalar1=1.0, scalar2=0.0,
                op0=ALU.mult, op1=ALU.add,
                accum_out=rs4[:, slot:slot + 1],
            )
            slot += 1
        elif vcs:
            src_ap = Es[:, j, vcs[0] * EXPW:(vcs[0] + 1) * EXPW]
            junk8 = jkp.tile([P, EXPW], FP8, tag="jk8", name=f"jk8{qi}")
            nc.vector.tensor_scalar(
                out=junk8, in0=src_ap,
                scalar1=1.0, scalar2=0.0,
                op0=ALU.mult, op1=ALU.add,
                accum_out=rs4[:, slot:slot + 1],
            )
            slot += 1
        nc.vector.reduce_sum(
            out=rsum, in_=rs4[:, 0:slot], axis=mybir.AxisListType.X)
        recf = rsp.tile([P, 1], F32, tag="recf", name=f"recf{qi}")
        nc.vector.reciprocal(out=recf, in_=rsum)
        nc.gpsimd.tensor_scalar(
            out=lt_of[pair][:, j, 8:9], in0=recf,
            scalar1=RECB_SCALE, scalar2=0.0,
            op0=ALU.mult, op1=ALU.add,
        )

    def emit_colsum(pair):
        Es = es_tiles.pop(pair)
        l = lt_of.pop(pair)
        NP = QTILES // 2

        def mk(m):
            def go():
                nc.tensor.matmul(
                    w_ps[m % 2],
                    l[:, :, 8 - m:16 - m],
                    Es[:, :, m * 512:(m + 1) * 512],
                    start=(pair == 0 and m < 2),
                    stop=(pair == NP - 1 and m >= 6),
                    perf_mode=mybir.MatmulPerfMode.DoubleRow,
                )
            return go
        for m in range(8):
            pending.append(mk(m))

    done = 0
    for qi in range(QTILES):
        emit_tile(qi)
        ready = (qi - COLSUM_LAG + 1) // 2
        while done < ready:
            emit_colsum(done)
            done += 1
    while done < QTILES // 2:
        emit_colsum(done)
        done += 1
    drain_pending(len(pending))

    nc.vector.tensor_copy(out=w_sb[:, 0:512], in_=w_ps[0])
    nc.scalar.activation(out=w_sb[:, 512:1024], in_=w_ps[1],
                         func=mybir.ActivationFunctionType.Copy)
    nc.sync.dma_start(out=w_d[:, :], in_=w_sb)


_NC_CACHE = None


def _build_nc():
    global _NC_CACHE
    if _NC_CACHE is None:
        from contextlib import ExitStack

        nc = bacc.Bacc("TRN2", target_bir_lowering=False, debug=False)
        with tile.TileContext(nc) as tc, ExitStack() as ctx:
            _emit(ctx, tc)
        nc.compile()
        _NC_CACHE = nc
    return _NC_CACHE


def _in_maps(inputs):
    import ml_dtypes

    e4 = ml_dtypes.float8_e4m3
    x = np.asarray(inputs["x"], dtype=np.float32)
    Wq = np.asarray(inputs["Wq"], dtype=np.float32)
    Wk = np.asarray(inputs["Wk"], dtype=np.float32)
    bq = np.asarray(inputs["bq"], dtype=np.float32)
    bk = np.asarray(inputs["bk"], dtype=np.float32)
    maps = []
    for c in range(N_CORES):
        b, h = divmod(c, 2)
        q = x[b, h * HALF:(h + 1) * HALF] @ Wq + bq          # [HALF, E]
        k = x[b] @ Wk + bk                                   # [S, E]
        # [E, n] -> [128, 2, n] with middle dim = E-chunk (eo)
        qt8 = np.ascontiguousarray(
            q.T.reshape(2, P, HALF).transpose(1, 0, 2)).astype(e4)
        kt8 = np.ascontiguousarray(
            k.T.reshape(2, P, S).transpose(1, 0, 2)).astype(e4)
        maps.append({"qt8": qt8, "kt8": kt8})
    return maps


def _combine(results, inputs):
    x = np.asarray(inputs["x"], dtype=np.float64)
    Wv = np.asarray(inputs["Wv"], dtype=np.float64)
    bv = np.asarray(inputs["bv"], dtype=np.float64)
    out = np.empty((B, 1, E), dtype=np.float32)
    for b in range(B):
        wt = (results[2 * b]["w"].astype(np.float64)
              + results[2 * b + 1]["w"].astype(np.float64))
        w = (wt[:, 0:512] + wt[:, 512:1024]).reshape(S) / RECB_SCALE
        u = w @ x[b]
        out[b, 0] = ((u / S) @ Wv + bv).astype(np.float32)
    return out


def kernel(**inputs):
    from concourse.bass_utils import run_bass_kernel_spmd

    nc = _build_nc()
    res = run_bass_kernel_spmd(nc, _in_maps(inputs), core_ids=list(range(N_CORES)))
    return _combine(res.results, inputs)
